# revision 3
# baseline (speedup 1.0000x reference)
"""Trainium2 Bass kernel for nn_Block_6725918785547 (dense_cnn encoder block).

Strategy: data-parallel over batch N=16 across 8 NeuronCores (2 images/core).
Each core runs the full block on its 2 images; no collectives.

Per-core pipeline (activations stay in SBUF; layout [C_partition, free]):
  conv_skip (1x1, f32r matmuls) -> pixel-norm (ones-matmul channel reduce +
  K=1 broadcast matmul) -> SiLU (ACT, writes zero-padded fp8 34x34 tiles) ->
  res0 3x3 conv as fp8 DoubleRow matmuls (2 ic-chunks packed per
  instruction, K=256 at 0.5 cyc/col) with a 2-term fp8 weight split
  (w ~ fp8(w*s) + fp8(w*s - fp8(w*s)), residual term restores ~bf16 weight
  accuracy; activation quantization dominates the error) -> SiLU*c -> fp8 ->
  res1 (same) -> residual add -> qkv 1x1 conv emitted TRANSPOSED
  ([pos, ch]) so per-head L2 norms are free-dim reductions -> normalize ->
  PE-transpose q,k back to [ch, pos] (bf16) -> per-head S' = k^T q in
  [kpos, qpos] layout -> exp on ACT -> P@V with an appended ones-column
  (M=65) so softmax row-sums come free -> normalize via K=1 broadcast
  matmul -> proj 1x1 conv -> residual add -> clip.

Weight normalization (weight-norm over fan-in), mp_silu/mp_sum scalar gains
and the tiny emb projection (c = emb @ W_emb^T * gain + 1, a [16,512]
per-channel scale) are folded on the host into the packed weights / scale
vectors; all O(N*C*H*W) compute runs on device.
"""

import numpy as np
import ml_dtypes

import concourse.bass as bass
import concourse.mybir as mybir
import concourse.tile as tile
from concourse import bacc
from concourse.bass_utils import run_bass_kernel_spmd
from concourse.masks import make_identity

P = 128
F32 = mybir.dt.float32
F32R = mybir.dt.float32r
BF16 = mybir.dt.bfloat16
F8 = mybir.dt.float8e4
E4 = ml_dtypes.float8_e4m3
DR = mybir.MatmulPerfMode.DoubleRow

EPS = 1e-4
MP_SILU_C = 0.596
RES_T = 0.3
ATTN_T = 0.3
CLIP = 256.0
HEADS = 8
CH = 64  # head dim

N_CORES = 8
IMG = 2          # images per core
HW = 1024        # 32*32
H = W = 32
PADW = 34        # padded spatial

_DEN_R = float(np.sqrt((1.0 - RES_T) ** 2 + RES_T**2))
C1 = (1.0 - RES_T) / _DEN_R     # residual: x-side gain
C2 = RES_T / _DEN_R             # residual: y-side gain
_DEN_A = float(np.sqrt((1.0 - ATTN_T) ** 2 + ATTN_T**2))
D1 = (1.0 - ATTN_T) / _DEN_A
D2 = ATTN_T / _DEN_A


# ---------------------------------------------------------------- device code

def build_kernel():
    nc = bacc.Bacc("TRN2", target_bir_lowering=False)

    xin_d = nc.dram_tensor("xin", [P, 2, IMG, HW], F32R, kind="ExternalInput")
    wskip_d = nc.dram_tensor("wskip", [P, 2, 512], F32R, kind="ExternalInput")
    w0_d = nc.dram_tensor("w0", [4, P, 2, 2, 9, 2, P], F8, kind="ExternalInput")
    w1_d = nc.dram_tensor("w1", [4, P, 2, 2, 9, 2, P], F8, kind="ExternalInput")
    wqkv_d = nc.dram_tensor("wqkv", [P, 4, 1536], F32R, kind="ExternalInput")
    wproj_d = nc.dram_tensor("wproj", [P, 4, 512], BF16, kind="ExternalInput")
    cvec_d = nc.dram_tensor("cvec", [P, 4, IMG], F32, kind="ExternalInput")
    ones_d = nc.dram_tensor("ones", [P, P], F32R, kind="ExternalInput")
    out_d = nc.dram_tensor("out", [P, 4, IMG, HW], F32, kind="ExternalOutput")

    with (
        nc.allow_low_precision(reason="deliberate f32r compute pipeline"),
        tile.TileContext(nc) as tc,
    ):
        with tc.tile_pool(name="persist", bufs=1) as pp:
            # persistent tiles: x2 (per img) and x3 (per img) rotate in one
            # 3-buf tag -- x3[img] reuses x2[other]'s slot after proj
            x2s = [
                pp.tile([P, 4, HW], F32R, tag="x2x3", bufs=2, name=f"x2_{i}")
                for i in range(IMG)
            ]
            wproj = pp.tile([P, 4, 512], BF16)
            cvec = pp.tile([P, 4, IMG], F32)
            onesT = pp.tile([P, P], F32R)
            ident = pp.tile([P, P], BF16)



            # ---------------- trunk: conv_skip, pixel norm, res convs -------
            with (
                tc.tile_pool(name="trunk", bufs=1) as tp,
                tc.tile_pool(name="tpsum", bufs=1, space="PSUM") as tps,
            ):
                wskip = tp.tile([P, 2, 512], F32R)
                nc.gpsimd.dma_start(wskip[:], wskip_d[:])

                x1s_map = {}
                for img in range(IMG):
                    xin = tp.tile([P, 2, HW], F32R, tag="xin", bufs=2, name="xin")
                    for cic in range(2):
                        for nt in range(2):
                            nc.sync.dma_start(
                                xin[:, cic, nt * 512 : (nt + 1) * 512],
                                xin_d[:, cic, img, nt * 512 : (nt + 1) * 512],
                            )
                    if img == 0:
                        nc.sync.dma_start(onesT[:], ones_d[:])
                        nc.sync.dma_start(cvec[:], cvec_d[:])
                        nc.sync.dma_start(wproj[:], wproj_d[:])
                        make_identity(nc, ident)
                    x1s = [
                        tp.tile([P, HW], F32R, tag="x1", bufs=8, name=f"x1_{c}")
                        for c in range(4)
                    ]
                    x1s_map[img] = x1s

                    # --- stage A: conv_skip (1x1, 256 -> 512) ---
                    for coc in range(4):
                        for nt in range(2):
                            ps = tps.tile([P, 512], F32, tag="conv", bufs=4,
                                          name="ps_skip")
                            for cic in range(2):
                                nc.tensor.matmul(
                                    ps[:],
                                    wskip[:, cic, coc * P : (coc + 1) * P],
                                    xin[:, cic, nt * 512 : (nt + 1) * 512],
                                    start=(cic == 0),
                                    stop=(cic == 1),
                                )
                            nc.any.tensor_copy(
                                x1s[coc][:, nt * 512 : (nt + 1) * 512], ps[:]
                            )

                for img in range(IMG):
                    x1s = x1s_map[img]
                    xpad0 = tp.tile([P, 4, PADW, PADW], F8, tag="xpad",
                                    bufs=2, name="xp0")
                    xpad1 = tp.tile([P, 4, PADW, PADW], F8, tag="xpad",
                                    bufs=2, name="xp1")
                    nc.gpsimd.memset(xpad0[:].bitcast(BF16), 0.0)
                    nc.gpsimd.memset(xpad1[:].bitcast(BF16), 0.0)

                    # --- stage B: pixel norm over channels ---
                    for nt in range(2):
                        ss = tps.tile([P, 512], F32, tag="ss", bufs=2, name="ps_ss")
                        for coc in range(4):
                            sq = tp.tile([P, 512], F32R, tag="sq", bufs=2, name="sq")
                            nc.vector.tensor_mul(
                                sq[:],
                                x1s[coc][:, nt * 512 : (nt + 1) * 512],
                                x1s[coc][:, nt * 512 : (nt + 1) * 512],
                            )
                            nc.tensor.matmul(
                                ss[0:1, :], onesT[:, 0:1], sq[:],
                                start=(coc == 0), stop=(coc == 3),
                            )
                        rs = tp.tile([P, 512], F32R, tag="rs", bufs=2, name="rs")
                        # rs = sqrt(ss/512); rs = 1/((rs+eps)/(D1*C1))
                        nc.scalar.activation(
                            rs[0:1, :], ss[0:1, :],
                            mybir.ActivationFunctionType.Sqrt, scale=1.0 / 512.0,
                        )
                        nc.vector.tensor_scalar(
                            rs[0:1, :], rs[0:1, :], EPS, 1.0 / (D1 * C1),
                            mybir.AluOpType.add, mybir.AluOpType.mult,
                        )
                        nc.vector.reciprocal(rs[0:1, :], rs[0:1, :])
                        rb = tps.tile([P, 512], F32, tag="rb", bufs=2, name="ps_rb")
                        nc.tensor.matmul(rb[:], onesT[0:1, :], rs[0:1, :],
                                         start=True, stop=True)
                        for coc in range(4):
                            nc.vector.tensor_mul(
                                x1s[coc][:, nt * 512 : (nt + 1) * 512],
                                x1s[coc][:, nt * 512 : (nt + 1) * 512],
                                rb[:],
                            )
                    # x1 now holds xn_s = D1*C1*normalize(conv_skip(x))

                    # --- stage C: silu -> xpad0 (res0 input) ---
                    for coc in range(4):
                        nc.scalar.activation(
                            xpad0[:, coc, 1:33, 1:33],
                            x1s[coc][:].rearrange("p (h w) -> p h w", h=H),
                            mybir.ActivationFunctionType.Silu,
                            scale=1.0 / (D1 * C1),
                        )

                    # --- stage D: res0 (fp8 DoubleRow, 2-term weights) ---
                    for coc in range(4):
                        w0c = tp.tile([P, 2, 2, 9, 2, P], F8, tag="wres",
                                      bufs=2, name="w0c")
                        nc.sync.dma_start(w0c[:], w0_d[coc])
                        for half in range(2):
                            h0 = half * 16
                            ps = tps.tile([P, 512], F32, tag="conv", bufs=4,
                                          name="ps_r0")
                            first = True
                            for term in range(2):
                                for pr in range(2):
                                    for tap in range(9):
                                        ky, kx = tap // 3, tap % 3
                                        nc.tensor.matmul(
                                            ps[:],
                                            w0c[:, term, pr, tap, :, :],
                                            xpad0[:, 2 * pr : 2 * pr + 2,
                                                  h0 + ky : h0 + ky + 16,
                                                  kx : kx + 32],
                                            start=first,
                                            stop=(term == 1 and pr == 1
                                                  and tap == 8),
                                            perf_mode=DR,
                                        )
                                        first = False
                            nc.scalar.activation(
                                xpad1[:, coc, 1 + h0 : 17 + h0, 1:33],
                                ps[:].rearrange("p (h w) -> p h w", h=16),
                                mybir.ActivationFunctionType.Silu,
                                scale=cvec[:, coc, img, None],
                            )

                    # --- stage E: res1 (fp8 DoubleRow, 2-term weights) ---
                    for coc in range(4):
                        w1c = tp.tile([P, 2, 2, 9, 2, P], F8, tag="wres",
                                      bufs=2, name="w1c")
                        nc.sync.dma_start(w1c[:], w1_d[coc])
                        for half in range(2):
                            h0 = half * 16
                            ps = tps.tile([P, 512], F32, tag="conv", bufs=4,
                                          name="ps_r1")
                            first = True
                            for term in range(2):
                                for pr in range(2):
                                    for tap in range(9):
                                        ky, kx = tap // 3, tap % 3
                                        nc.tensor.matmul(
                                            ps[:],
                                            w1c[:, term, pr, tap, :, :],
                                            xpad1[:, 2 * pr : 2 * pr + 2,
                                                  h0 + ky : h0 + ky + 16,
                                                  kx : kx + 32],
                                            start=first,
                                            stop=(term == 1 and pr == 1
                                                  and tap == 8),
                                            perf_mode=DR,
                                        )
                                        first = False
                            sl = slice(half * 512, half * 512 + 512)
                            nc.vector.scalar_tensor_tensor(
                                x2s[img][:, coc, sl], ps[:], 1.0 / S1R,
                                x1s[coc][:, sl],
                                mybir.AluOpType.mult, mybir.AluOpType.add,
                            )

            # ---------------- attention ------------------------------------
            with (
                tc.tile_pool(name="attn", bufs=1) as ap,
                tc.tile_pool(name="apsum", bufs=1, space="PSUM") as aps,
            ):
                wqkv = ap.tile([P, 4, 1536], F32R)
                for ncol in range(3):
                    nc.sync.dma_start(
                        wqkv[:, :, ncol * 512 : (ncol + 1) * 512],
                        wqkv_d[:, :, ncol * 512 : (ncol + 1) * 512],
                    )

                prepped = {}  # img -> (vT, qh, kh)

                def qkv_prep(img):
                    # qkv conv, transposed out: qkvT[pos, col], col=s*512+h*64+c
                    qkvTs = [
                        ap.tile([P, 1536], BF16, tag="qkvT", bufs=9,
                                name=f"qkvT_{c}")
                        for c in range(8)
                    ]
                    for pc in range(8):
                        qkvT = qkvTs[pc]
                        psb = aps.tile([P, 1024], F32, tag="big", bufs=3,
                                       name="ps_qkv")
                        pss = aps.tile([P, 512], F32, tag="small", bufs=2,
                                       name="ps_qkv2")
                        for ncol in range(3):
                            ps = (
                                psb[:, (ncol % 2) * 512 : (ncol % 2) * 512 + 512]
                                if ncol < 2
                                else pss[:]
                            )
                            for cic in range(4):
                                nc.tensor.matmul(
                                    ps,
                                    x2s[img][:, cic, pc * P : (pc + 1) * P],
                                    wqkv[:, cic, ncol * 512 : (ncol + 1) * 512],
                                    start=(cic == 0),
                                    stop=(cic == 3),
                                )
                        nc.any.tensor_copy(qkvT[:, 0:1024], psb[:])
                        nc.any.tensor_copy(qkvT[:, 1024:1536], pss[:])

                    # per-(pos, s, h) L2 norms over the 64 head channels;
                    # two half-tiles so the finalize doesn't gate all chunks
                    nrms = [
                        ap.tile([P, 4, 24], F32, tag="nrm", bufs=4,
                                name=f"nrm_{hf}")
                        for hf in range(2)
                    ]
                    for hf in range(2):
                        for pp_ in range(4):
                            pc = hf * 4 + pp_
                            sqv = ap.tile([P, 1536], BF16, tag="sqv", bufs=2,
                                          name="sqv")
                            nc.vector.tensor_mul(
                                sqv[:], qkvTs[pc][:], qkvTs[pc][:]
                            )
                            nc.vector.tensor_reduce(
                                nrms[hf][:, pp_, :],
                                sqv[:].rearrange("p (s c) -> p s c", c=CH),
                                axis=mybir.AxisListType.X,
                                op=mybir.AluOpType.add,
                            )
                        # r = 1/(eps + sqrt(nrm/64))
                        nc.scalar.activation(
                            nrms[hf][:], nrms[hf][:],
                            mybir.ActivationFunctionType.Sqrt, scale=1.0 / CH,
                        )
                        nc.vector.tensor_scalar_add(nrms[hf][:], nrms[hf][:], EPS)
                        nc.vector.reciprocal(nrms[hf][:], nrms[hf][:])

                    # normalize per pos-chunk into per-chunk tiles (so the PE
                    # transposes pipeline behind them); v -> vT_aug + ones col
                    vT = ap.tile([P, 8, HEADS, CH + 1], BF16, tag="vT", bufs=2,
                                 name="vT")
                    nc.vector.memset(vT[:, :, :, CH], 1.0)
                    qhq = [
                        ap.tile([P, 4, 512], BF16, tag="qhq", bufs=4,
                                name=f"qhq_{t}")
                        for t in range(2)
                    ]
                    khc = [
                        ap.tile([P, 4, P], BF16, tag="khc", bufs=16,
                                name=f"khc_{t}")
                        for t in range(8)
                    ]
                    for pc in range(8):
                        qkn = ap.tile([P, 16, CH], BF16, tag="qkn", bufs=6,
                                      name="qkn")
                        qkvT4 = qkvTs[pc][:].rearrange("p (s c) -> p s c", c=CH)
                        nc.vector.tensor_mul(
                            qkn[:],
                            qkvT4[:, 0:16, :],
                            nrms[pc // 4][:, pc % 4, 0:16, None]
                            .to_broadcast([P, 16, CH]),
                        )
                        nc.vector.tensor_mul(
                            vT[:, pc, :, 0:CH],
                            qkvT4[:, 16:24, :],
                            nrms[pc // 4][:, pc % 4, 16:24, None]
                            .to_broadcast([P, HEADS, CH]),
                        )
                        qkn2 = qkn[:].rearrange("p a c -> p (a c)")
                        for off, outap in (
                            (0, qhq[pc // 4][:, :, (pc % 4) * P : (pc % 4 + 1) * P]),
                            (512, khc[pc][:, :, :]),
                        ):
                            pst = aps.tile([P, 512], BF16, tag="small", bufs=2,
                                           name="ps_tp")
                            for hc in range(4):
                                nc.tensor.transpose(
                                    pst[:, hc * P : (hc + 1) * P],
                                    qkn2[:, off + hc * P : off + (hc + 1) * P],
                                    ident[:],
                                )
                            nc.vector.tensor_copy(
                                outap,
                                pst[:].rearrange("p (a c) -> p a c", a=4),
                            )
                    prepped[img] = (vT, qhq, khc)

                oall_map = {}

                def attn_units(img, hcs):
                    vT, qhq, khc = prepped[img]
                    if img not in oall_map:
                        oall_map[img] = [
                            ap.tile([P, HW], BF16, tag="oall", bufs=8,
                                    name=f"oall_{c}")
                            for c in range(4)
                        ]
                    oalls = oall_map[img]
                    # head pairs emitted adjacently: even head on PE rows 0-63,
                    # odd head on rows 64-127 run concurrently (row groups)
                    for hc in hcs:
                        for qt in range(2):
                            # per-group P' tiles so PV pipelines behind exp
                            pqg = [
                                [
                                    ap.tile([P, 2, 512], BF16, tag="pq",
                                            bufs=12, name="pqg")
                                    for _ in range(4)
                                ]
                                for _ in range(2)
                            ]
                            for g in range(4):
                                pbs = [
                                    aps.tile([P, 1024], F32, tag="big", bufs=3,
                                             name="ps_s")
                                    for _ in range(2)
                                ]
                                for i in range(2):
                                    kc = g * 2 + i
                                    for a in range(2):
                                        hp = a * CH
                                        nc.tensor.matmul(
                                            pbs[a][:, i * 512 : (i + 1) * 512],
                                            khc[kc][hp : hp + CH, hc, :],
                                            qhq[qt][hp : hp + CH, hc, :],
                                            start=True, stop=True,
                                        )
                                for a in range(2):
                                    nc.scalar.activation(
                                        pqg[a][g][:], pbs[a][:],
                                        mybir.ActivationFunctionType.Exp,
                                        scale=1.0 / 8.0,
                                    )
                            for a in range(2):
                                h = 2 * hc + a
                                hp = a * CH
                                pso = aps.tile([P, 512], F32, tag="small", bufs=2,
                                               name="ps_o")
                                for g in range(4):
                                    for i in range(2):
                                        kc = g * 2 + i
                                        nc.tensor.matmul(
                                            pso[0 : CH + 1, :],
                                            vT[:, kc, h, :],
                                            pqg[a][g][:, i, :],
                                            start=(kc == 0),
                                            stop=(kc == 7),
                                        )
                                otmp = ap.tile([P, 512], F32, tag="otmp", bufs=2,
                                               name="otmp")
                                nc.vector.tensor_copy(
                                    otmp[0 : CH + 1, :], pso[0 : CH + 1, :]
                                )
                                rr = ap.tile([P, 512], F32R, tag="rr", bufs=2,
                                             name="rr")
                                nc.vector.reciprocal(
                                    rr[0:1, :], otmp[CH : CH + 1, :]
                                )
                                psr = aps.tile([P, 512], F32, tag="small", bufs=2,
                                               name="ps_r")
                                nc.tensor.matmul(
                                    psr[0:CH, :], onesT[0:1, 0:CH], rr[0:1, :],
                                    start=True, stop=True,
                                )
                                nc.vector.tensor_mul(
                                    oalls[hc][hp : hp + CH,
                                              qt * 512 : (qt + 1) * 512],
                                    otmp[0:CH, :],
                                    psr[0:CH, :],
                                )

                def attn_proj(img):
                    oalls = oall_map[img]
                    # proj + residual + clip + store (per img, in-place on x2)
                    for coc in range(4):
                        for nt in range(2):
                            ps = aps.tile([P, 512], F32, tag="small", bufs=2,
                                          name="ps_p")
                            for cic in range(4):
                                nc.tensor.matmul(
                                    ps[:],
                                    wproj[:, cic, coc * P : (coc + 1) * P],
                                    oalls[cic][:, nt * 512 : (nt + 1) * 512],
                                    start=(cic == 0),
                                    stop=(cic == 3),
                                )
                            sl = slice(nt * 512, nt * 512 + 512)
                            nc.vector.tensor_add(
                                x2s[img][:, coc, sl], ps[:],
                                x2s[img][:, coc, sl],
                            )
                        nc.vector.tensor_scalar(
                            x2s[img][:, coc, :],
                            x2s[img][:, coc, :], CLIP, -CLIP,
                            mybir.AluOpType.min, mybir.AluOpType.max,
                        )
                        nc.sync.dma_start(
                            out_d[:, coc, img, :],
                            x2s[img][:, coc, :].bitcast(F32),
                        )

                qkv_prep(0)
                attn_units(0, (0, 1))
                qkv_prep(1)
                attn_units(0, (2, 3))
                attn_proj(0)
                attn_units(1, (0, 1, 2, 3))
                attn_proj(1)

    nc.compile()
    return nc


# ---------------------------------------------------------------- host side

def _normalize_w(w):
    w = w.astype(np.float64)
    axes = tuple(range(1, w.ndim))
    norm = np.sqrt((w**2).sum(axis=axes, keepdims=True))
    alpha = np.sqrt(norm.size / w.size)
    return w / (EPS + alpha * norm)


def _pack_weights(w_skip, w_res0, w_res1, w_emb, w_qkv, w_proj, emb_gain, emb):
    # conv_skip: fan=256, gain=1
    ws = _normalize_w(w_skip[:, :, 0, 0]) / np.sqrt(256.0)
    wskip = ws.T.reshape(2, P, 512).transpose(1, 0, 2)  # [128, 2, 512]

    # res convs: fp8 2-term split (hi + residual), DoubleRow layout
    w0 = (_normalize_w(w_res0.reshape(512, -1)).reshape(512, 512, 3, 3)
          / np.sqrt(512 * 9.0) / MP_SILU_C)
    w0p = _pack_res_f8(w0, S0R)
    w1 = (_normalize_w(w_res1.reshape(512, -1)).reshape(512, 512, 3, 3)
          / np.sqrt(512 * 9.0) * (D1 * C2 / MP_SILU_C))
    w1p = _pack_res_f8(w1, S1R)

    # qkv: reorder rows to [s, h, c]
    wq = _normalize_w(w_qkv[:, :, 0, 0]) / np.sqrt(512.0)  # [1536, 512]
    s_idx, h_idx, c_idx = np.meshgrid(
        np.arange(3), np.arange(HEADS), np.arange(CH), indexing="ij"
    )
    perm = ((h_idx * CH + c_idx) * 3 + s_idx).reshape(-1)
    wqp = wq[perm]  # rows ordered s*512 + h*64 + c
    wqkvT = wqp.T.reshape(4, P, 1536).transpose(1, 0, 2)  # [128, 4, 1536]

    # proj: fold D2
    wp = _normalize_w(w_proj[:, :, 0, 0]) / np.sqrt(512.0) * D2
    wprojT = wp.T.reshape(4, P, 512).transpose(1, 0, 2)  # [128, 4, 512]

    # emb scale: c = emb @ w_emb_n.T * gain + 1, pre-divided by the res0
    # fp8 weight scale (applied in the stage-D silu)
    we = _normalize_w(w_emb) * (float(emb_gain) / np.sqrt(1024.0))
    c = (emb.astype(np.float64) @ we.T + 1.0) / S0R  # [16, 512]

    return (
        np.ascontiguousarray(wskip).astype(np.float32),
        w0p,
        w1p,
        np.ascontiguousarray(wqkvT).astype(np.float32),
        np.ascontiguousarray(wprojT).astype(ml_dtypes.bfloat16),
        np.ascontiguousarray(c).astype(np.float32),
    )


S0R = 32.0
S1R = 128.0


def _pack_res_f8(weff, scale):
    ws = (weff * scale).astype(np.float32)
    hi = ws.astype(E4)
    lo = (ws - hi.astype(np.float32)).astype(E4)
    out = np.zeros([4, P, 2, 2, 9, 2, P], E4)
    for term, srcw in enumerate((hi, lo)):
        s = srcw.reshape(4, P, 2, 2, P, 3, 3)   # occ ocP p t icP ky kx
        s = s.transpose(0, 4, 2, 5, 6, 3, 1)    # occ icP p ky kx t ocP
        out[:, :, term] = s.reshape(4, P, 2, 9, 2, P)
    return np.ascontiguousarray(out)


_NC_CACHE = None


def kernel(x, emb, w_skip, w_res0, w_res1, w_emb, w_qkv, w_proj, emb_gain):
    global _NC_CACHE
    if _NC_CACHE is None:
        _NC_CACHE = build_kernel()
    nc = _NC_CACHE

    x = np.asarray(x, dtype=np.float32)
    wskip, w0p, w1p, wqkvT, wprojT, c = _pack_weights(
        np.asarray(w_skip, np.float32),
        np.asarray(w_res0, np.float32),
        np.asarray(w_res1, np.float32),
        np.asarray(w_emb, np.float32),
        np.asarray(w_qkv, np.float32),
        np.asarray(w_proj, np.float32),
        np.asarray(emb_gain, np.float32),
        np.asarray(emb, np.float32),
    )
    ones = np.ones((P, P), dtype=np.float32)

    in_maps = []
    for core in range(N_CORES):
        xi = x[core * IMG : (core + 1) * IMG].reshape(IMG, 2, P, HW)
        xi = np.ascontiguousarray(xi.transpose(2, 1, 0, 3))  # [128, 2, IMG, HW]
        ci = c[core * IMG : (core + 1) * IMG]  # [IMG, 512]
        ci = np.ascontiguousarray(ci.T.reshape(4, P, IMG).transpose(1, 0, 2))
        in_maps.append(
            {
                "xin": xi,
                "wskip": wskip,
                "w0": w0p,
                "w1": w1p,
                "wqkv": wqkvT,
                "wproj": wprojT,
                "cvec": ci,
                "ones": ones,
            }
        )

    # The axon-tunneled device occasionally reports a transient
    # NRT_EXEC_UNIT_UNRECOVERABLE on the first execution after a fresh
    # process start; a retry succeeds.
    import time as _time

    res = None
    for attempt in range(5):
        try:
            res = run_bass_kernel_spmd(nc, in_maps, core_ids=list(range(N_CORES)))
            break
        except Exception:
            if attempt == 4:
                raise
            _time.sleep(2.0 * (attempt + 1))
    outs = []
    for core in range(N_CORES):
        o = res.results[core]["out"]  # [128, 4, IMG, HW]
        o = o.transpose(2, 1, 0, 3).reshape(IMG, 512, H, W)
        outs.append(o)
    return np.concatenate(outs, axis=0).astype(np.float32)



# revision 4
# speedup vs baseline: 1.0329x; 1.0329x over previous
"""Trainium2 Bass kernel for nn_Block_6725918785547 (dense_cnn encoder block).

Strategy: data-parallel over batch N=16 across 8 NeuronCores (2 images/core).
Each core runs the full block on its 2 images; no collectives.

Per-core pipeline (activations stay in SBUF; layout [C_partition, free]):
  conv_skip (1x1, f32r matmuls) -> pixel-norm (ones-matmul channel reduce +
  K=1 broadcast matmul) -> SiLU (ACT, writes zero-padded fp8 34x34 tiles) ->
  res0 3x3 conv as fp8 DoubleRow matmuls (2 ic-chunks packed per
  instruction, K=256 at 0.5 cyc/col) with a 2-term fp8 weight split
  (w ~ fp8(w*s) + fp8(w*s - fp8(w*s)), residual term restores ~bf16 weight
  accuracy; activation quantization dominates the error) -> SiLU*c -> fp8 ->
  res1 (same) -> residual add -> qkv 1x1 conv emitted TRANSPOSED
  ([pos, ch]) so per-head L2 norms are free-dim reductions -> normalize ->
  PE-transpose q,k back to [ch, pos] (bf16) -> per-head S' = k^T q in
  [kpos, qpos] layout -> exp on ACT -> P@V with an appended ones-column
  (M=65) so softmax row-sums come free -> normalize via K=1 broadcast
  matmul -> proj 1x1 conv -> residual add -> clip.

Weight normalization (weight-norm over fan-in), mp_silu/mp_sum scalar gains
and the tiny emb projection (c = emb @ W_emb^T * gain + 1, a [16,512]
per-channel scale) are folded on the host into the packed weights / scale
vectors; all O(N*C*H*W) compute runs on device.
"""

import numpy as np
import ml_dtypes

import concourse.bass as bass
import concourse.mybir as mybir
import concourse.tile as tile
from concourse import bacc
from concourse.bass_utils import run_bass_kernel_spmd
from concourse.masks import make_identity

P = 128
F32 = mybir.dt.float32
F32R = mybir.dt.float32r
BF16 = mybir.dt.bfloat16
F8 = mybir.dt.float8e4
E4 = ml_dtypes.float8_e4m3
DR = mybir.MatmulPerfMode.DoubleRow

EPS = 1e-4
MP_SILU_C = 0.596
RES_T = 0.3
ATTN_T = 0.3
CLIP = 256.0
HEADS = 8
CH = 64  # head dim

N_CORES = 8
IMG = 2          # images per core
HW = 1024        # 32*32
H = W = 32
PADW = 34        # padded spatial

_DEN_R = float(np.sqrt((1.0 - RES_T) ** 2 + RES_T**2))
C1 = (1.0 - RES_T) / _DEN_R     # residual: x-side gain
C2 = RES_T / _DEN_R             # residual: y-side gain
_DEN_A = float(np.sqrt((1.0 - ATTN_T) ** 2 + ATTN_T**2))
D1 = (1.0 - ATTN_T) / _DEN_A
D2 = ATTN_T / _DEN_A


# ---------------------------------------------------------------- device code

def build_kernel():
    nc = bacc.Bacc("TRN2", target_bir_lowering=False)

    xin_d = nc.dram_tensor("xin", [P, 2, IMG, HW], F32R, kind="ExternalInput")
    wskip_d = nc.dram_tensor("wskip", [P, 2, 512], F32R, kind="ExternalInput")
    w0_d = nc.dram_tensor("w0", [4, P, 2, 2, 9, 2, P], F8, kind="ExternalInput")
    w1_d = nc.dram_tensor("w1", [4, P, 2, 2, 9, 2, P], F8, kind="ExternalInput")
    wqkv_d = nc.dram_tensor("wqkv", [P, 4, 1536], F8, kind="ExternalInput")
    wproj_d = nc.dram_tensor("wproj", [P, 4, 512], BF16, kind="ExternalInput")
    cvec_d = nc.dram_tensor("cvec", [P, 4, IMG], F32, kind="ExternalInput")
    ones_d = nc.dram_tensor("ones", [P, P], F32R, kind="ExternalInput")
    out_d = nc.dram_tensor("out", [P, 4, IMG, HW], F32, kind="ExternalOutput")

    with (
        nc.allow_low_precision(reason="deliberate f32r compute pipeline"),
        tile.TileContext(nc) as tc,
    ):
        with tc.tile_pool(name="persist", bufs=1) as pp:
            # persistent tiles: x2 (per img) and x3 (per img) rotate in one
            # 3-buf tag -- x3[img] reuses x2[other]'s slot after proj
            x2s = [
                pp.tile([P, 4, HW], F32R, tag="x2x3", bufs=2, name=f"x2_{i}")
                for i in range(IMG)
            ]
            wproj = pp.tile([P, 4, 512], BF16)
            cvec = pp.tile([P, 4, IMG], F32)
            onesT = pp.tile([P, P], F32R)
            ident = pp.tile([P, P], BF16)



            # ---------------- trunk: conv_skip, pixel norm, res convs -------
            with (
                tc.tile_pool(name="trunk", bufs=1) as tp,
                tc.tile_pool(name="tpsum", bufs=1, space="PSUM") as tps,
            ):
                wskip = tp.tile([P, 2, 512], F32R)
                nc.gpsimd.dma_start(wskip[:], wskip_d[:])

                x1s_map = {}
                for img in range(IMG):
                    xin = tp.tile([P, 2, HW], F32R, tag="xin", bufs=2, name="xin")
                    for cic in range(2):
                        for nt in range(2):
                            nc.sync.dma_start(
                                xin[:, cic, nt * 512 : (nt + 1) * 512],
                                xin_d[:, cic, img, nt * 512 : (nt + 1) * 512],
                            )
                    if img == 0:
                        nc.sync.dma_start(onesT[:], ones_d[:])
                        nc.sync.dma_start(cvec[:], cvec_d[:])
                        nc.sync.dma_start(wproj[:], wproj_d[:])
                        make_identity(nc, ident)
                    x1s = [
                        tp.tile([P, HW], F32R, tag="x1", bufs=8, name=f"x1_{c}")
                        for c in range(4)
                    ]
                    x1s_map[img] = x1s

                    # --- stage A: conv_skip (1x1, 256 -> 512) ---
                    for coc in range(4):
                        for nt in range(2):
                            ps = tps.tile([P, 512], F32, tag="conv", bufs=4,
                                          name="ps_skip")
                            for cic in range(2):
                                nc.tensor.matmul(
                                    ps[:],
                                    wskip[:, cic, coc * P : (coc + 1) * P],
                                    xin[:, cic, nt * 512 : (nt + 1) * 512],
                                    start=(cic == 0),
                                    stop=(cic == 1),
                                )
                            nc.any.tensor_copy(
                                x1s[coc][:, nt * 512 : (nt + 1) * 512], ps[:]
                            )

                for img in range(IMG):
                    x1s = x1s_map[img]
                    xpad0 = tp.tile([P, 4, PADW, PADW], F8, tag="xpad",
                                    bufs=2, name="xp0")
                    xpad1 = tp.tile([P, 4, PADW, PADW], F8, tag="xpad",
                                    bufs=2, name="xp1")
                    nc.gpsimd.memset(xpad0[:].bitcast(BF16), 0.0)
                    nc.gpsimd.memset(xpad1[:].bitcast(BF16), 0.0)

                    # --- stage B: pixel norm over channels ---
                    for nt in range(2):
                        ss = tps.tile([P, 512], F32, tag="ss", bufs=2, name="ps_ss")
                        for coc in range(4):
                            sq = tp.tile([P, 512], F32R, tag="sq", bufs=2, name="sq")
                            nc.vector.tensor_mul(
                                sq[:],
                                x1s[coc][:, nt * 512 : (nt + 1) * 512],
                                x1s[coc][:, nt * 512 : (nt + 1) * 512],
                            )
                            nc.tensor.matmul(
                                ss[0:1, :], onesT[:, 0:1], sq[:],
                                start=(coc == 0), stop=(coc == 3),
                            )
                        rs = tp.tile([P, 512], F32R, tag="rs", bufs=2, name="rs")
                        # rs = sqrt(ss/512); rs = 1/((rs+eps)/(D1*C1))
                        nc.scalar.activation(
                            rs[0:1, :], ss[0:1, :],
                            mybir.ActivationFunctionType.Sqrt, scale=1.0 / 512.0,
                        )
                        nc.vector.tensor_scalar(
                            rs[0:1, :], rs[0:1, :], EPS, 1.0 / (D1 * C1),
                            mybir.AluOpType.add, mybir.AluOpType.mult,
                        )
                        nc.vector.reciprocal(rs[0:1, :], rs[0:1, :])
                        rb = tps.tile([P, 512], F32, tag="rb", bufs=2, name="ps_rb")
                        nc.tensor.matmul(rb[:], onesT[0:1, :], rs[0:1, :],
                                         start=True, stop=True)
                        for coc in range(4):
                            nc.vector.tensor_mul(
                                x1s[coc][:, nt * 512 : (nt + 1) * 512],
                                x1s[coc][:, nt * 512 : (nt + 1) * 512],
                                rb[:],
                            )
                    # x1 now holds xn_s = D1*C1*normalize(conv_skip(x))

                    # --- stage C: silu -> xpad0 (res0 input) ---
                    for coc in range(4):
                        nc.scalar.activation(
                            xpad0[:, coc, 1:33, 1:33],
                            x1s[coc][:].rearrange("p (h w) -> p h w", h=H),
                            mybir.ActivationFunctionType.Silu,
                            scale=1.0 / (D1 * C1),
                        )

                    # --- stage D: res0 (fp8 DoubleRow, 2-term weights) ---
                    for coc in range(4):
                        w0c = tp.tile([P, 2, 2, 9, 2, P], F8, tag="wres",
                                      bufs=2, name="w0c")
                        nc.sync.dma_start(w0c[:], w0_d[coc])
                        for half in range(2):
                            h0 = half * 16
                            ps = tps.tile([P, 512], F32, tag="conv", bufs=4,
                                          name="ps_r0")
                            first = True
                            for term in range(2):
                                for pr in range(2):
                                    for tap in range(9):
                                        ky, kx = tap // 3, tap % 3
                                        nc.tensor.matmul(
                                            ps[:],
                                            w0c[:, term, pr, tap, :, :],
                                            xpad0[:, 2 * pr : 2 * pr + 2,
                                                  h0 + ky : h0 + ky + 16,
                                                  kx : kx + 32],
                                            start=first,
                                            stop=(term == 1 and pr == 1
                                                  and tap == 8),
                                            perf_mode=DR,
                                        )
                                        first = False
                            nc.scalar.activation(
                                xpad1[:, coc, 1 + h0 : 17 + h0, 1:33],
                                ps[:].rearrange("p (h w) -> p h w", h=16),
                                mybir.ActivationFunctionType.Silu,
                                scale=cvec[:, coc, img, None],
                            )

                    # --- stage E: res1 (fp8 DoubleRow, 2-term weights) ---
                    for coc in range(4):
                        w1c = tp.tile([P, 2, 2, 9, 2, P], F8, tag="wres",
                                      bufs=2, name="w1c")
                        nc.sync.dma_start(w1c[:], w1_d[coc])
                        for half in range(2):
                            h0 = half * 16
                            ps = tps.tile([P, 512], F32, tag="conv", bufs=4,
                                          name="ps_r1")
                            first = True
                            for term in range(2):
                                for pr in range(2):
                                    for tap in range(9):
                                        ky, kx = tap // 3, tap % 3
                                        nc.tensor.matmul(
                                            ps[:],
                                            w1c[:, term, pr, tap, :, :],
                                            xpad1[:, 2 * pr : 2 * pr + 2,
                                                  h0 + ky : h0 + ky + 16,
                                                  kx : kx + 32],
                                            start=first,
                                            stop=(term == 1 and pr == 1
                                                  and tap == 8),
                                            perf_mode=DR,
                                        )
                                        first = False
                            sl = slice(half * 512, half * 512 + 512)
                            nc.vector.scalar_tensor_tensor(
                                x2s[img][:, coc, sl], ps[:], 1.0 / S1R,
                                x1s[coc][:, sl],
                                mybir.AluOpType.mult, mybir.AluOpType.add,
                            )

            # ---------------- attention ------------------------------------
            with (
                tc.tile_pool(name="attn", bufs=1) as ap,
                tc.tile_pool(name="apsum", bufs=1, space="PSUM") as aps,
            ):
                wqkv = ap.tile([P, 4, 1536], F8)
                for ncol in range(3):
                    nc.sync.dma_start(
                        wqkv[:, :, ncol * 512 : (ncol + 1) * 512],
                        wqkv_d[:, :, ncol * 512 : (ncol + 1) * 512],
                    )

                prepped = {}  # img -> (vT, qh, kh)

                def qkv_prep(img):
                    # qkv conv as fp8 DoubleRow (stationary x2 in fp8, two
                    # ic-chunks packed per matmul); transposed out:
                    # qkvT[pos, col], col=s*512+h*64+c
                    x28 = ap.tile([P, 4, HW], F8, tag="x28", bufs=2,
                                  name="x28")
                    for c in range(4):
                        nc.gpsimd.tensor_copy(x28[:, c, :], x2s[img][:, c, :])
                    qkvTs = [
                        ap.tile([P, 1536], BF16, tag="qkvT", bufs=9,
                                name=f"qkvT_{c}")
                        for c in range(8)
                    ]
                    for pc in range(8):
                        qkvT = qkvTs[pc]
                        psb = aps.tile([P, 1024], F32, tag="big", bufs=3,
                                       name="ps_qkv")
                        pss = aps.tile([P, 512], F32, tag="small", bufs=2,
                                       name="ps_qkv2")
                        for ncol in range(3):
                            ps = (
                                psb[:, (ncol % 2) * 512 : (ncol % 2) * 512 + 512]
                                if ncol < 2
                                else pss[:]
                            )
                            for pr in range(2):
                                nc.tensor.matmul(
                                    ps,
                                    x28[:, 2 * pr : 2 * pr + 2,
                                        pc * P : (pc + 1) * P],
                                    wqkv[:, 2 * pr : 2 * pr + 2,
                                         ncol * 512 : (ncol + 1) * 512],
                                    start=(pr == 0),
                                    stop=(pr == 1),
                                    perf_mode=DR,
                                )
                        # evict with 1/SQ8 scale on ACT (keeps DVE free)
                        nc.scalar.mul(qkvT[:, 0:1024], psb[:], 1.0 / SQ8)
                        nc.scalar.mul(qkvT[:, 1024:1536], pss[:], 1.0 / SQ8)

                    # per-(pos, s, h) L2 norms over the 64 head channels;
                    # two half-tiles so the finalize doesn't gate all chunks
                    nrms = [
                        ap.tile([P, 4, 24], F32, tag="nrm", bufs=4,
                                name=f"nrm_{hf}")
                        for hf in range(2)
                    ]
                    for hf in range(2):
                        for pp_ in range(4):
                            pc = hf * 4 + pp_
                            sqv = ap.tile([P, 1536], BF16, tag="sqv", bufs=2,
                                          name="sqv")
                            nc.vector.tensor_mul(
                                sqv[:], qkvTs[pc][:], qkvTs[pc][:]
                            )
                            nc.vector.tensor_reduce(
                                nrms[hf][:, pp_, :],
                                sqv[:].rearrange("p (s c) -> p s c", c=CH),
                                axis=mybir.AxisListType.X,
                                op=mybir.AluOpType.add,
                            )
                        # r = 1/(eps + sqrt(nrm/64))
                        nc.scalar.activation(
                            nrms[hf][:], nrms[hf][:],
                            mybir.ActivationFunctionType.Sqrt, scale=1.0 / CH,
                        )
                        nc.vector.tensor_scalar_add(nrms[hf][:], nrms[hf][:], EPS)
                        nc.vector.reciprocal(nrms[hf][:], nrms[hf][:])

                    # normalize per pos-chunk into per-chunk tiles (so the PE
                    # transposes pipeline behind them); v -> vT_aug + ones col
                    vT = ap.tile([P, 8, HEADS, CH + 1], BF16, tag="vT", bufs=2,
                                 name="vT")
                    nc.vector.memset(vT[:, :, :, CH], 1.0)
                    qhq = [
                        ap.tile([P, 4, 512], BF16, tag="qhq", bufs=4,
                                name=f"qhq_{t}")
                        for t in range(2)
                    ]
                    khc = [
                        ap.tile([P, 4, P], BF16, tag="khc", bufs=16,
                                name=f"khc_{t}")
                        for t in range(8)
                    ]
                    for pc in range(8):
                        qkn = ap.tile([P, 16, CH], BF16, tag="qkn", bufs=6,
                                      name="qkn")
                        qkvT4 = qkvTs[pc][:].rearrange("p (s c) -> p s c", c=CH)
                        nc.vector.tensor_mul(
                            qkn[:],
                            qkvT4[:, 0:16, :],
                            nrms[pc // 4][:, pc % 4, 0:16, None]
                            .to_broadcast([P, 16, CH]),
                        )
                        nc.vector.tensor_mul(
                            vT[:, pc, :, 0:CH],
                            qkvT4[:, 16:24, :],
                            nrms[pc // 4][:, pc % 4, 16:24, None]
                            .to_broadcast([P, HEADS, CH]),
                        )
                        qkn2 = qkn[:].rearrange("p a c -> p (a c)")
                        for off, outap in (
                            (0, qhq[pc // 4][:, :, (pc % 4) * P : (pc % 4 + 1) * P]),
                            (512, khc[pc][:, :, :]),
                        ):
                            pst = aps.tile([P, 512], BF16, tag="small", bufs=2,
                                           name="ps_tp")
                            for hc in range(4):
                                nc.tensor.transpose(
                                    pst[:, hc * P : (hc + 1) * P],
                                    qkn2[:, off + hc * P : off + (hc + 1) * P],
                                    ident[:],
                                )
                            nc.vector.tensor_copy(
                                outap,
                                pst[:].rearrange("p (a c) -> p a c", a=4),
                            )
                    prepped[img] = (vT, qhq, khc)

                oall_map = {}

                def attn_units(img, hcs):
                    vT, qhq, khc = prepped[img]
                    if img not in oall_map:
                        oall_map[img] = [
                            ap.tile([P, HW], BF16, tag="oall", bufs=8,
                                    name=f"oall_{c}")
                            for c in range(4)
                        ]
                    oalls = oall_map[img]
                    # head pairs emitted adjacently: even head on PE rows 0-63,
                    # odd head on rows 64-127 run concurrently (row groups)
                    for hc in hcs:
                        for qt in range(2):
                            # per-group P' tiles so PV pipelines behind exp
                            pqg = [
                                [
                                    ap.tile([P, 2, 512], BF16, tag="pq",
                                            bufs=12, name="pqg")
                                    for _ in range(4)
                                ]
                                for _ in range(2)
                            ]
                            for g in range(4):
                                pbs = [
                                    aps.tile([P, 1024], F32, tag="big", bufs=3,
                                             name="ps_s")
                                    for _ in range(2)
                                ]
                                for i in range(2):
                                    kc = g * 2 + i
                                    for a in range(2):
                                        hp = a * CH
                                        nc.tensor.matmul(
                                            pbs[a][:, i * 512 : (i + 1) * 512],
                                            khc[kc][hp : hp + CH, hc, :],
                                            qhq[qt][hp : hp + CH, hc, :],
                                            start=True, stop=True,
                                        )
                                for a in range(2):
                                    nc.scalar.activation(
                                        pqg[a][g][:], pbs[a][:],
                                        mybir.ActivationFunctionType.Exp,
                                        scale=1.0 / 8.0,
                                    )
                            for a in range(2):
                                h = 2 * hc + a
                                hp = a * CH
                                pso = aps.tile([P, 512], F32, tag="small", bufs=2,
                                               name="ps_o")
                                for g in range(4):
                                    for i in range(2):
                                        kc = g * 2 + i
                                        nc.tensor.matmul(
                                            pso[0 : CH + 1, :],
                                            vT[:, kc, h, :],
                                            pqg[a][g][:, i, :],
                                            start=(kc == 0),
                                            stop=(kc == 7),
                                        )
                                otmp = ap.tile([P, 512], F32, tag="otmp", bufs=2,
                                               name="otmp")
                                nc.vector.tensor_copy(
                                    otmp[0 : CH + 1, :], pso[0 : CH + 1, :]
                                )
                                rr = ap.tile([P, 512], F32R, tag="rr", bufs=2,
                                             name="rr")
                                nc.vector.reciprocal(
                                    rr[0:1, :], otmp[CH : CH + 1, :]
                                )
                                psr = aps.tile([P, 512], F32, tag="small", bufs=2,
                                               name="ps_r")
                                nc.tensor.matmul(
                                    psr[0:CH, :], onesT[0:1, 0:CH], rr[0:1, :],
                                    start=True, stop=True,
                                )
                                nc.vector.tensor_mul(
                                    oalls[hc][hp : hp + CH,
                                              qt * 512 : (qt + 1) * 512],
                                    otmp[0:CH, :],
                                    psr[0:CH, :],
                                )

                def attn_proj(img):
                    oalls = oall_map[img]
                    # proj + residual + clip + store (per img, in-place on x2)
                    for coc in range(4):
                        for nt in range(2):
                            ps = aps.tile([P, 512], F32, tag="small", bufs=2,
                                          name="ps_p")
                            for cic in range(4):
                                nc.tensor.matmul(
                                    ps[:],
                                    wproj[:, cic, coc * P : (coc + 1) * P],
                                    oalls[cic][:, nt * 512 : (nt + 1) * 512],
                                    start=(cic == 0),
                                    stop=(cic == 3),
                                )
                            sl = slice(nt * 512, nt * 512 + 512)
                            nc.vector.tensor_add(
                                x2s[img][:, coc, sl], ps[:],
                                x2s[img][:, coc, sl],
                            )
                        nc.vector.tensor_scalar(
                            x2s[img][:, coc, :],
                            x2s[img][:, coc, :], CLIP, -CLIP,
                            mybir.AluOpType.min, mybir.AluOpType.max,
                        )
                        nc.sync.dma_start(
                            out_d[:, coc, img, :],
                            x2s[img][:, coc, :].bitcast(F32),
                        )

                qkv_prep(0)
                attn_units(0, (0, 1))
                qkv_prep(1)
                attn_units(0, (2, 3))
                attn_proj(0)
                attn_units(1, (0, 1, 2, 3))
                attn_proj(1)

    nc.compile()
    return nc


# ---------------------------------------------------------------- host side

def _normalize_w(w):
    w = w.astype(np.float64)
    axes = tuple(range(1, w.ndim))
    norm = np.sqrt((w**2).sum(axis=axes, keepdims=True))
    alpha = np.sqrt(norm.size / w.size)
    return w / (EPS + alpha * norm)


def _pack_weights(w_skip, w_res0, w_res1, w_emb, w_qkv, w_proj, emb_gain, emb):
    # conv_skip: fan=256, gain=1
    ws = _normalize_w(w_skip[:, :, 0, 0]) / np.sqrt(256.0)
    wskip = ws.T.reshape(2, P, 512).transpose(1, 0, 2)  # [128, 2, 512]

    # res convs: fp8 2-term split (hi + residual), DoubleRow layout
    w0 = (_normalize_w(w_res0.reshape(512, -1)).reshape(512, 512, 3, 3)
          / np.sqrt(512 * 9.0) / MP_SILU_C)
    w0p = _pack_res_f8(w0, S0R)
    w1 = (_normalize_w(w_res1.reshape(512, -1)).reshape(512, 512, 3, 3)
          / np.sqrt(512 * 9.0) * (D1 * C2 / MP_SILU_C))
    w1p = _pack_res_f8(w1, S1R)

    # qkv: reorder rows to [s, h, c]
    wq = _normalize_w(w_qkv[:, :, 0, 0]) / np.sqrt(512.0)  # [1536, 512]
    s_idx, h_idx, c_idx = np.meshgrid(
        np.arange(3), np.arange(HEADS), np.arange(CH), indexing="ij"
    )
    perm = ((h_idx * CH + c_idx) * 3 + s_idx).reshape(-1)
    wqp = wq[perm]  # rows ordered s*512 + h*64 + c
    wqkvT = (wqp.T.reshape(4, P, 1536).transpose(1, 0, 2) * SQ8)  # [128, 4, 1536]

    # proj: fold D2
    wp = _normalize_w(w_proj[:, :, 0, 0]) / np.sqrt(512.0) * D2
    wprojT = wp.T.reshape(4, P, 512).transpose(1, 0, 2)  # [128, 4, 512]

    # emb scale: c = emb @ w_emb_n.T * gain + 1, pre-divided by the res0
    # fp8 weight scale (applied in the stage-D silu)
    we = _normalize_w(w_emb) * (float(emb_gain) / np.sqrt(1024.0))
    c = (emb.astype(np.float64) @ we.T + 1.0) / S0R  # [16, 512]

    return (
        np.ascontiguousarray(wskip).astype(np.float32),
        w0p,
        w1p,
        np.ascontiguousarray(wqkvT).astype(E4),
        np.ascontiguousarray(wprojT).astype(ml_dtypes.bfloat16),
        np.ascontiguousarray(c).astype(np.float32),
    )


S0R = 32.0
S1R = 128.0
SQ8 = 16.0


def _pack_res_f8(weff, scale):
    ws = (weff * scale).astype(np.float32)
    hi = ws.astype(E4)
    lo = (ws - hi.astype(np.float32)).astype(E4)
    out = np.zeros([4, P, 2, 2, 9, 2, P], E4)
    for term, srcw in enumerate((hi, lo)):
        s = srcw.reshape(4, P, 2, 2, P, 3, 3)   # occ ocP p t icP ky kx
        s = s.transpose(0, 4, 2, 5, 6, 3, 1)    # occ icP p ky kx t ocP
        out[:, :, term] = s.reshape(4, P, 2, 9, 2, P)
    return np.ascontiguousarray(out)


_NC_CACHE = None


def kernel(x, emb, w_skip, w_res0, w_res1, w_emb, w_qkv, w_proj, emb_gain):
    global _NC_CACHE
    if _NC_CACHE is None:
        _NC_CACHE = build_kernel()
    nc = _NC_CACHE

    x = np.asarray(x, dtype=np.float32)
    wskip, w0p, w1p, wqkvT, wprojT, c = _pack_weights(
        np.asarray(w_skip, np.float32),
        np.asarray(w_res0, np.float32),
        np.asarray(w_res1, np.float32),
        np.asarray(w_emb, np.float32),
        np.asarray(w_qkv, np.float32),
        np.asarray(w_proj, np.float32),
        np.asarray(emb_gain, np.float32),
        np.asarray(emb, np.float32),
    )
    ones = np.ones((P, P), dtype=np.float32)

    in_maps = []
    for core in range(N_CORES):
        xi = x[core * IMG : (core + 1) * IMG].reshape(IMG, 2, P, HW)
        xi = np.ascontiguousarray(xi.transpose(2, 1, 0, 3))  # [128, 2, IMG, HW]
        ci = c[core * IMG : (core + 1) * IMG]  # [IMG, 512]
        ci = np.ascontiguousarray(ci.T.reshape(4, P, IMG).transpose(1, 0, 2))
        in_maps.append(
            {
                "xin": xi,
                "wskip": wskip,
                "w0": w0p,
                "w1": w1p,
                "wqkv": wqkvT,
                "wproj": wprojT,
                "cvec": ci,
                "ones": ones,
            }
        )

    # The axon-tunneled device occasionally reports a transient
    # NRT_EXEC_UNIT_UNRECOVERABLE on the first execution after a fresh
    # process start; a retry succeeds.
    import time as _time

    res = None
    for attempt in range(5):
        try:
            res = run_bass_kernel_spmd(nc, in_maps, core_ids=list(range(N_CORES)))
            break
        except Exception:
            if attempt == 4:
                raise
            _time.sleep(2.0 * (attempt + 1))
    outs = []
    for core in range(N_CORES):
        o = res.results[core]["out"]  # [128, 4, IMG, HW]
        o = o.transpose(2, 1, 0, 3).reshape(IMG, 512, H, W)
        outs.append(o)
    return np.concatenate(outs, axis=0).astype(np.float32)



# revision 6
# speedup vs baseline: 1.0438x; 1.0105x over previous
"""Trainium2 Bass kernel for nn_Block_6725918785547 (dense_cnn encoder block).

Strategy: data-parallel over batch N=16 across 8 NeuronCores (2 images/core).
Each core runs the full block on its 2 images; no collectives.

Per-core pipeline (activations stay in SBUF; layout [C_partition, free]):
  conv_skip (1x1, f32r matmuls) -> pixel-norm (ones-matmul channel reduce +
  K=1 broadcast matmul) -> SiLU (ACT, writes zero-padded fp8 34x34 tiles) ->
  res0 3x3 conv as fp8 DoubleRow matmuls (2 ic-chunks packed per
  instruction, K=256 at 0.5 cyc/col) with a 2-term fp8 weight split
  (w ~ fp8(w*s) + fp8(w*s - fp8(w*s)), residual term restores ~bf16 weight
  accuracy; activation quantization dominates the error) -> SiLU*c -> fp8 ->
  res1 (same) -> residual add -> qkv 1x1 conv emitted TRANSPOSED
  ([pos, ch]) so per-head L2 norms are free-dim reductions -> normalize ->
  PE-transpose q,k back to [ch, pos] (bf16) -> per-head S' = k^T q in
  [kpos, qpos] layout -> exp on ACT -> P@V with an appended ones-column
  (M=65) so softmax row-sums come free -> normalize via K=1 broadcast
  matmul -> proj 1x1 conv -> residual add -> clip.

Weight normalization (weight-norm over fan-in), mp_silu/mp_sum scalar gains
and the tiny emb projection (c = emb @ W_emb^T * gain + 1, a [16,512]
per-channel scale) are folded on the host into the packed weights / scale
vectors; all O(N*C*H*W) compute runs on device.
"""

import numpy as np
import ml_dtypes

import concourse.bass as bass
import concourse.mybir as mybir
import concourse.tile as tile
from concourse import bacc
from concourse.bass_utils import run_bass_kernel_spmd
from concourse.masks import make_identity

P = 128
F32 = mybir.dt.float32
F32R = mybir.dt.float32r
BF16 = mybir.dt.bfloat16
F8 = mybir.dt.float8e4
E4 = ml_dtypes.float8_e4m3
DR = mybir.MatmulPerfMode.DoubleRow

EPS = 1e-4
MP_SILU_C = 0.596
RES_T = 0.3
ATTN_T = 0.3
CLIP = 256.0
HEADS = 8
CH = 64  # head dim

N_CORES = 8
IMG = 2          # images per core
HW = 1024        # 32*32
H = W = 32
PADW = 34        # padded spatial

_DEN_R = float(np.sqrt((1.0 - RES_T) ** 2 + RES_T**2))
C1 = (1.0 - RES_T) / _DEN_R     # residual: x-side gain
C2 = RES_T / _DEN_R             # residual: y-side gain
_DEN_A = float(np.sqrt((1.0 - ATTN_T) ** 2 + ATTN_T**2))
D1 = (1.0 - ATTN_T) / _DEN_A
D2 = ATTN_T / _DEN_A


# ---------------------------------------------------------------- device code

def build_kernel():
    nc = bacc.Bacc("TRN2", target_bir_lowering=False)

    xin_d = nc.dram_tensor("xin", [P, 2, IMG, HW], F32R, kind="ExternalInput")
    wskip_d = nc.dram_tensor("wskip", [P, 2, 512], F32R, kind="ExternalInput")
    w0_d = nc.dram_tensor("w0", [4, P, 2, 2, 9, 2, P], F8, kind="ExternalInput")
    w1_d = nc.dram_tensor("w1", [4, P, 2, 2, 9, 2, P], F8, kind="ExternalInput")
    wqkv_d = nc.dram_tensor("wqkv", [P, 4, 1536], F8, kind="ExternalInput")
    wproj_d = nc.dram_tensor("wproj", [P, 4, 512], F8, kind="ExternalInput")
    cvec_d = nc.dram_tensor("cvec", [P, 4, IMG], F32, kind="ExternalInput")
    ones_d = nc.dram_tensor("ones", [P, P], F32R, kind="ExternalInput")
    out_d = nc.dram_tensor("out", [P, 4, IMG, HW], F32, kind="ExternalOutput")

    with (
        nc.allow_low_precision(reason="deliberate f32r compute pipeline"),
        tile.TileContext(nc) as tc,
    ):
        with tc.tile_pool(name="persist", bufs=1) as pp:
            # persistent tiles: x2 (per img) and x3 (per img) rotate in one
            # 3-buf tag -- x3[img] reuses x2[other]'s slot after proj
            x2s = [
                pp.tile([P, 4, HW], F32R, tag="x2x3", bufs=2, name=f"x2_{i}")
                for i in range(IMG)
            ]
            wproj = pp.tile([P, 4, 512], F8)
            cvec = pp.tile([P, 4, IMG], F32)
            onesT = pp.tile([P, P], F32R)
            ident = pp.tile([P, P], BF16)



            # ---------------- trunk: conv_skip, pixel norm, res convs -------
            with (
                tc.tile_pool(name="trunk", bufs=1) as tp,
                tc.tile_pool(name="tpsum", bufs=1, space="PSUM") as tps,
            ):
                wskip = tp.tile([P, 2, 512], F32R)
                nc.gpsimd.dma_start(wskip[:], wskip_d[:])

                x1s_map = {}
                for img in range(IMG):
                    xin = tp.tile([P, 2, HW], F32R, tag="xin", bufs=2, name="xin")
                    for cic in range(2):
                        for nt in range(2):
                            nc.sync.dma_start(
                                xin[:, cic, nt * 512 : (nt + 1) * 512],
                                xin_d[:, cic, img, nt * 512 : (nt + 1) * 512],
                            )
                    if img == 0:
                        nc.sync.dma_start(onesT[:], ones_d[:])
                        nc.sync.dma_start(cvec[:], cvec_d[:])
                        nc.sync.dma_start(wproj[:], wproj_d[:])
                        make_identity(nc, ident)
                    x1s = [
                        tp.tile([P, HW], F32R, tag="x1", bufs=8, name=f"x1_{c}")
                        for c in range(4)
                    ]
                    x1s_map[img] = x1s

                    # --- stage A: conv_skip (1x1, 256 -> 512) ---
                    for coc in range(4):
                        for nt in range(2):
                            ps = tps.tile([P, 512], F32, tag="conv", bufs=4,
                                          name="ps_skip")
                            for cic in range(2):
                                nc.tensor.matmul(
                                    ps[:],
                                    wskip[:, cic, coc * P : (coc + 1) * P],
                                    xin[:, cic, nt * 512 : (nt + 1) * 512],
                                    start=(cic == 0),
                                    stop=(cic == 1),
                                )
                            nc.any.tensor_copy(
                                x1s[coc][:, nt * 512 : (nt + 1) * 512], ps[:]
                            )

                for img in range(IMG):
                    x1s = x1s_map[img]
                    xpad0 = tp.tile([P, 4, PADW, PADW], F8, tag="xpad",
                                    bufs=2, name="xp0")
                    xpad1 = tp.tile([P, 4, PADW, PADW], F8, tag="xpad",
                                    bufs=2, name="xp1")
                    nc.gpsimd.memset(xpad0[:].bitcast(BF16), 0.0)
                    nc.gpsimd.memset(xpad1[:].bitcast(BF16), 0.0)

                    # --- stage B: pixel norm over channels ---
                    for nt in range(2):
                        ss = tps.tile([P, 512], F32, tag="ss", bufs=2, name="ps_ss")
                        for coc in range(4):
                            sq = tp.tile([P, 512], F32R, tag="sq", bufs=2, name="sq")
                            nc.vector.tensor_mul(
                                sq[:],
                                x1s[coc][:, nt * 512 : (nt + 1) * 512],
                                x1s[coc][:, nt * 512 : (nt + 1) * 512],
                            )
                            nc.tensor.matmul(
                                ss[0:1, :], onesT[:, 0:1], sq[:],
                                start=(coc == 0), stop=(coc == 3),
                            )
                        rs = tp.tile([P, 512], F32R, tag="rs", bufs=2, name="rs")
                        # rs = sqrt(ss/512); rs = 1/((rs+eps)/(D1*C1))
                        nc.scalar.activation(
                            rs[0:1, :], ss[0:1, :],
                            mybir.ActivationFunctionType.Sqrt, scale=1.0 / 512.0,
                        )
                        nc.vector.tensor_scalar(
                            rs[0:1, :], rs[0:1, :], EPS, 1.0 / (D1 * C1),
                            mybir.AluOpType.add, mybir.AluOpType.mult,
                        )
                        nc.vector.reciprocal(rs[0:1, :], rs[0:1, :])
                        rb = tps.tile([P, 512], F32, tag="rb", bufs=2, name="ps_rb")
                        nc.tensor.matmul(rb[:], onesT[0:1, :], rs[0:1, :],
                                         start=True, stop=True)
                        for coc in range(4):
                            nc.vector.tensor_mul(
                                x1s[coc][:, nt * 512 : (nt + 1) * 512],
                                x1s[coc][:, nt * 512 : (nt + 1) * 512],
                                rb[:],
                            )
                    # x1 now holds xn_s = D1*C1*normalize(conv_skip(x))

                    # --- stage C: silu -> xpad0 (res0 input) ---
                    for coc in range(4):
                        nc.scalar.activation(
                            xpad0[:, coc, 1:33, 1:33],
                            x1s[coc][:].rearrange("p (h w) -> p h w", h=H),
                            mybir.ActivationFunctionType.Silu,
                            scale=1.0 / (D1 * C1),
                        )

                    # --- stage D: res0 (fp8 DoubleRow, 2-term weights) ---
                    for coc in range(4):
                        w0c = tp.tile([P, 2, 2, 9, 2, P], F8, tag="wres",
                                      bufs=2, name="w0c")
                        nc.sync.dma_start(w0c[:], w0_d[coc])
                        for half in range(2):
                            h0 = half * 16
                            ps = tps.tile([P, 512], F32, tag="conv", bufs=4,
                                          name="ps_r0")
                            first = True
                            for term in range(2):
                                for pr in range(2):
                                    for tap in range(9):
                                        ky, kx = tap // 3, tap % 3
                                        nc.tensor.matmul(
                                            ps[:],
                                            w0c[:, term, pr, tap, :, :],
                                            xpad0[:, 2 * pr : 2 * pr + 2,
                                                  h0 + ky : h0 + ky + 16,
                                                  kx : kx + 32],
                                            start=first,
                                            stop=(term == 1 and pr == 1
                                                  and tap == 8),
                                            perf_mode=DR,
                                        )
                                        first = False
                            nc.scalar.activation(
                                xpad1[:, coc, 1 + h0 : 17 + h0, 1:33],
                                ps[:].rearrange("p (h w) -> p h w", h=16),
                                mybir.ActivationFunctionType.Silu,
                                scale=cvec[:, coc, img, None],
                            )

                    # --- stage E: res1 (fp8 DoubleRow, 2-term weights) ---
                    for coc in range(4):
                        w1c = tp.tile([P, 2, 2, 9, 2, P], F8, tag="wres",
                                      bufs=2, name="w1c")
                        nc.sync.dma_start(w1c[:], w1_d[coc])
                        for half in range(2):
                            h0 = half * 16
                            ps = tps.tile([P, 512], F32, tag="conv", bufs=4,
                                          name="ps_r1")
                            first = True
                            for term in range(2):
                                for pr in range(2):
                                    for tap in range(9):
                                        ky, kx = tap // 3, tap % 3
                                        nc.tensor.matmul(
                                            ps[:],
                                            w1c[:, term, pr, tap, :, :],
                                            xpad1[:, 2 * pr : 2 * pr + 2,
                                                  h0 + ky : h0 + ky + 16,
                                                  kx : kx + 32],
                                            start=first,
                                            stop=(term == 1 and pr == 1
                                                  and tap == 8),
                                            perf_mode=DR,
                                        )
                                        first = False
                            sl = slice(half * 512, half * 512 + 512)
                            nc.vector.scalar_tensor_tensor(
                                x2s[img][:, coc, sl], ps[:], 1.0 / S1R,
                                x1s[coc][:, sl],
                                mybir.AluOpType.mult, mybir.AluOpType.add,
                            )

            # ---------------- attention ------------------------------------
            with (
                tc.tile_pool(name="attn", bufs=1) as ap,
                tc.tile_pool(name="apsum", bufs=1, space="PSUM") as aps,
            ):
                wqkv = ap.tile([P, 4, 1536], F8)
                for ncol in range(3):
                    nc.sync.dma_start(
                        wqkv[:, :, ncol * 512 : (ncol + 1) * 512],
                        wqkv_d[:, :, ncol * 512 : (ncol + 1) * 512],
                    )

                prepped = {}  # img -> (vT, qh, kh)

                def qkv_prep(img):
                    # qkv conv as fp8 DoubleRow (stationary x2 in fp8, two
                    # ic-chunks packed per matmul); transposed out:
                    # qkvT[pos, col], col=s*512+h*64+c
                    x28 = ap.tile([P, 4, HW], F8, tag="x28", bufs=2,
                                  name="x28")
                    for c in range(4):
                        nc.gpsimd.tensor_copy(x28[:, c, :], x2s[img][:, c, :])
                    qkvTs = [
                        ap.tile([P, 1536], BF16, tag="qkvT", bufs=9,
                                name=f"qkvT_{c}")
                        for c in range(8)
                    ]
                    for pc in range(8):
                        qkvT = qkvTs[pc]
                        psb = aps.tile([P, 1024], F32, tag="big", bufs=3,
                                       name="ps_qkv")
                        pss = aps.tile([P, 512], F32, tag="small", bufs=2,
                                       name="ps_qkv2")
                        for ncol in range(3):
                            ps = (
                                psb[:, (ncol % 2) * 512 : (ncol % 2) * 512 + 512]
                                if ncol < 2
                                else pss[:]
                            )
                            for pr in range(2):
                                nc.tensor.matmul(
                                    ps,
                                    x28[:, 2 * pr : 2 * pr + 2,
                                        pc * P : (pc + 1) * P],
                                    wqkv[:, 2 * pr : 2 * pr + 2,
                                         ncol * 512 : (ncol + 1) * 512],
                                    start=(pr == 0),
                                    stop=(pr == 1),
                                    perf_mode=DR,
                                )
                        # evict with 1/SQ8 scale on ACT (keeps DVE free)
                        nc.scalar.mul(qkvT[:, 0:1024], psb[:], 1.0 / SQ8)
                        nc.scalar.mul(qkvT[:, 1024:1536], pss[:], 1.0 / SQ8)

                    # per-(pos, s, h) L2 norms over the 64 head channels;
                    # two half-tiles so the finalize doesn't gate all chunks
                    nrms = [
                        ap.tile([P, 4, 24], F32, tag="nrm", bufs=4,
                                name=f"nrm_{hf}")
                        for hf in range(2)
                    ]
                    for hf in range(2):
                        for pp_ in range(4):
                            pc = hf * 4 + pp_
                            sqv = ap.tile([P, 1536], BF16, tag="sqv", bufs=2,
                                          name="sqv")
                            nc.vector.tensor_mul(
                                sqv[:], qkvTs[pc][:], qkvTs[pc][:]
                            )
                            nc.vector.tensor_reduce(
                                nrms[hf][:, pp_, :],
                                sqv[:].rearrange("p (s c) -> p s c", c=CH),
                                axis=mybir.AxisListType.X,
                                op=mybir.AluOpType.add,
                            )
                        # r = 1/(eps + sqrt(nrm/64))
                        nc.scalar.activation(
                            nrms[hf][:], nrms[hf][:],
                            mybir.ActivationFunctionType.Sqrt, scale=1.0 / CH,
                        )
                        nc.vector.tensor_scalar_add(nrms[hf][:], nrms[hf][:], EPS)
                        nc.vector.reciprocal(nrms[hf][:], nrms[hf][:])

                    # normalize per pos-chunk into per-chunk tiles (so the PE
                    # transposes pipeline behind them); v -> vT_aug + ones col
                    vT = ap.tile([P, 8, HEADS, CH + 1], BF16, tag="vT", bufs=2,
                                 name="vT")
                    nc.vector.memset(vT[:, :, :, CH], 1.0)
                    qhq = [
                        ap.tile([P, 4, 512], BF16, tag="qhq", bufs=4,
                                name=f"qhq_{t}")
                        for t in range(2)
                    ]
                    khc = [
                        ap.tile([P, 4, P], BF16, tag="khc", bufs=16,
                                name=f"khc_{t}")
                        for t in range(8)
                    ]
                    for pc in range(8):
                        qkn = ap.tile([P, 16, CH], BF16, tag="qkn", bufs=6,
                                      name="qkn")
                        qkvT4 = qkvTs[pc][:].rearrange("p (s c) -> p s c", c=CH)
                        nc.vector.tensor_mul(
                            qkn[:],
                            qkvT4[:, 0:16, :],
                            nrms[pc // 4][:, pc % 4, 0:16, None]
                            .to_broadcast([P, 16, CH]),
                        )
                        nc.vector.tensor_mul(
                            vT[:, pc, :, 0:CH],
                            qkvT4[:, 16:24, :],
                            nrms[pc // 4][:, pc % 4, 16:24, None]
                            .to_broadcast([P, HEADS, CH]),
                        )
                        qkn2 = qkn[:].rearrange("p a c -> p (a c)")
                        for off, outap in (
                            (0, qhq[pc // 4][:, :, (pc % 4) * P : (pc % 4 + 1) * P]),
                            (512, khc[pc][:, :, :]),
                        ):
                            pst = aps.tile([P, 512], BF16, tag="small", bufs=2,
                                           name="ps_tp")
                            for hc in range(4):
                                nc.tensor.transpose(
                                    pst[:, hc * P : (hc + 1) * P],
                                    qkn2[:, off + hc * P : off + (hc + 1) * P],
                                    ident[:],
                                )
                            nc.vector.tensor_copy(
                                outap,
                                pst[:].rearrange("p (a c) -> p a c", a=4),
                            )
                    prepped[img] = (vT, qhq, khc)

                oall_map = {}

                def attn_units(img, hcs):
                    vT, qhq, khc = prepped[img]
                    if img not in oall_map:
                        oall_map[img] = ap.tile([P, 4, HW], F8, tag="oall",
                                                bufs=2, name="oall")
                    oall = oall_map[img]
                    # head pairs emitted adjacently: even head on PE rows 0-63,
                    # odd head on rows 64-127 run concurrently (row groups)
                    for hc in hcs:
                        for qt in range(2):
                            # per-group P' tiles so PV pipelines behind exp
                            pqg = [
                                [
                                    ap.tile([P, 2, 512], BF16, tag="pq",
                                            bufs=12, name="pqg")
                                    for _ in range(4)
                                ]
                                for _ in range(2)
                            ]
                            for g in range(4):
                                pbs = [
                                    aps.tile([P, 1024], F32, tag="big", bufs=3,
                                             name="ps_s")
                                    for _ in range(2)
                                ]
                                for i in range(2):
                                    kc = g * 2 + i
                                    for a in range(2):
                                        hp = a * CH
                                        nc.tensor.matmul(
                                            pbs[a][:, i * 512 : (i + 1) * 512],
                                            khc[kc][hp : hp + CH, hc, :],
                                            qhq[qt][hp : hp + CH, hc, :],
                                            start=True, stop=True,
                                        )
                                for a in range(2):
                                    nc.scalar.activation(
                                        pqg[a][g][:], pbs[a][:],
                                        mybir.ActivationFunctionType.Exp,
                                        scale=1.0 / 8.0,
                                    )
                            for a in range(2):
                                h = 2 * hc + a
                                hp = a * CH
                                pso = aps.tile([P, 512], F32, tag="small", bufs=2,
                                               name="ps_o")
                                for g in range(4):
                                    for i in range(2):
                                        kc = g * 2 + i
                                        nc.tensor.matmul(
                                            pso[0 : CH + 1, :],
                                            vT[:, kc, h, :],
                                            pqg[a][g][:, i, :],
                                            start=(kc == 0),
                                            stop=(kc == 7),
                                        )
                                otmp = ap.tile([P, 512], F32, tag="otmp", bufs=2,
                                               name="otmp")
                                nc.vector.tensor_copy(
                                    otmp[0 : CH + 1, :], pso[0 : CH + 1, :]
                                )
                                rr = ap.tile([P, 512], F32R, tag="rr", bufs=2,
                                             name="rr")
                                nc.vector.reciprocal(
                                    rr[0:1, :], otmp[CH : CH + 1, :]
                                )
                                psr = aps.tile([P, 512], F32, tag="small", bufs=2,
                                               name="ps_r")
                                nc.tensor.matmul(
                                    psr[0:CH, :], onesT[0:1, 0:CH], rr[0:1, :],
                                    start=True, stop=True,
                                )
                                nc.vector.tensor_mul(
                                    oall[hp : hp + CH, hc,
                                         qt * 512 : (qt + 1) * 512],
                                    otmp[0:CH, :],
                                    psr[0:CH, :],
                                )

                def attn_proj(img):
                    oall = oall_map[img]
                    # proj (fp8 DoubleRow) + residual + clip + store
                    for coc in range(4):
                        for nt in range(2):
                            ps = aps.tile([P, 512], F32, tag="small", bufs=2,
                                          name="ps_p")
                            for pr in range(2):
                                nc.tensor.matmul(
                                    ps[:],
                                    wproj[:, 2 * pr : 2 * pr + 2,
                                          coc * P : (coc + 1) * P],
                                    oall[:, 2 * pr : 2 * pr + 2,
                                         nt * 512 : (nt + 1) * 512],
                                    start=(pr == 0),
                                    stop=(pr == 1),
                                    perf_mode=DR,
                                )
                            sl = slice(nt * 512, nt * 512 + 512)
                            nc.vector.scalar_tensor_tensor(
                                x2s[img][:, coc, sl], ps[:], 1.0 / SP8,
                                x2s[img][:, coc, sl],
                                mybir.AluOpType.mult, mybir.AluOpType.add,
                            )
                        nc.vector.tensor_scalar(
                            x2s[img][:, coc, :],
                            x2s[img][:, coc, :], CLIP, -CLIP,
                            mybir.AluOpType.min, mybir.AluOpType.max,
                        )
                        nc.sync.dma_start(
                            out_d[:, coc, img, :],
                            x2s[img][:, coc, :].bitcast(F32),
                        )

                qkv_prep(0)
                attn_units(0, (0, 1))
                qkv_prep(1)
                attn_units(0, (2, 3))
                attn_proj(0)
                attn_units(1, (0, 1, 2, 3))
                attn_proj(1)

    nc.compile()
    return nc


# ---------------------------------------------------------------- host side

def _normalize_w(w):
    w = w.astype(np.float64)
    axes = tuple(range(1, w.ndim))
    norm = np.sqrt((w**2).sum(axis=axes, keepdims=True))
    alpha = np.sqrt(norm.size / w.size)
    return w / (EPS + alpha * norm)


def _pack_weights(w_skip, w_res0, w_res1, w_emb, w_qkv, w_proj, emb_gain, emb):
    # conv_skip: fan=256, gain=1
    ws = _normalize_w(w_skip[:, :, 0, 0]) / np.sqrt(256.0)
    wskip = ws.T.reshape(2, P, 512).transpose(1, 0, 2)  # [128, 2, 512]

    # res convs: fp8 2-term split (hi + residual), DoubleRow layout
    w0 = (_normalize_w(w_res0.reshape(512, -1)).reshape(512, 512, 3, 3)
          / np.sqrt(512 * 9.0) / MP_SILU_C)
    w0p = _pack_res_f8(w0, S0R)
    w1 = (_normalize_w(w_res1.reshape(512, -1)).reshape(512, 512, 3, 3)
          / np.sqrt(512 * 9.0) * (D1 * C2 / MP_SILU_C))
    w1p = _pack_res_f8(w1, S1R)

    # qkv: reorder rows to [s, h, c]
    wq = _normalize_w(w_qkv[:, :, 0, 0]) / np.sqrt(512.0)  # [1536, 512]
    s_idx, h_idx, c_idx = np.meshgrid(
        np.arange(3), np.arange(HEADS), np.arange(CH), indexing="ij"
    )
    perm = ((h_idx * CH + c_idx) * 3 + s_idx).reshape(-1)
    wqp = wq[perm]  # rows ordered s*512 + h*64 + c
    wqkvT = (wqp.T.reshape(4, P, 1536).transpose(1, 0, 2) * SQ8)  # [128, 4, 1536]

    # proj: fold D2
    wp = _normalize_w(w_proj[:, :, 0, 0]) / np.sqrt(512.0) * D2 * SP8
    wprojT = wp.T.reshape(4, P, 512).transpose(1, 0, 2)  # [128, 4, 512]

    # emb scale: c = emb @ w_emb_n.T * gain + 1, pre-divided by the res0
    # fp8 weight scale (applied in the stage-D silu)
    we = _normalize_w(w_emb) * (float(emb_gain) / np.sqrt(1024.0))
    c = (emb.astype(np.float64) @ we.T + 1.0) / S0R  # [16, 512]

    return (
        np.ascontiguousarray(wskip).astype(np.float32),
        w0p,
        w1p,
        np.ascontiguousarray(wqkvT).astype(E4),
        np.ascontiguousarray(wprojT).astype(E4),
        np.ascontiguousarray(c).astype(np.float32),
    )


S0R = 32.0
S1R = 128.0
SQ8 = 16.0
SP8 = 64.0


def _pack_res_f8(weff, scale):
    ws = (weff * scale).astype(np.float32)
    hi = ws.astype(E4)
    lo = (ws - hi.astype(np.float32)).astype(E4)
    out = np.zeros([4, P, 2, 2, 9, 2, P], E4)
    for term, srcw in enumerate((hi, lo)):
        s = srcw.reshape(4, P, 2, 2, P, 3, 3)   # occ ocP p t icP ky kx
        s = s.transpose(0, 4, 2, 5, 6, 3, 1)    # occ icP p ky kx t ocP
        out[:, :, term] = s.reshape(4, P, 2, 9, 2, P)
    return np.ascontiguousarray(out)


_NC_CACHE = None


def kernel(x, emb, w_skip, w_res0, w_res1, w_emb, w_qkv, w_proj, emb_gain):
    global _NC_CACHE
    if _NC_CACHE is None:
        _NC_CACHE = build_kernel()
    nc = _NC_CACHE

    x = np.asarray(x, dtype=np.float32)
    wskip, w0p, w1p, wqkvT, wprojT, c = _pack_weights(
        np.asarray(w_skip, np.float32),
        np.asarray(w_res0, np.float32),
        np.asarray(w_res1, np.float32),
        np.asarray(w_emb, np.float32),
        np.asarray(w_qkv, np.float32),
        np.asarray(w_proj, np.float32),
        np.asarray(emb_gain, np.float32),
        np.asarray(emb, np.float32),
    )
    ones = np.ones((P, P), dtype=np.float32)

    in_maps = []
    for core in range(N_CORES):
        xi = x[core * IMG : (core + 1) * IMG].reshape(IMG, 2, P, HW)
        xi = np.ascontiguousarray(xi.transpose(2, 1, 0, 3))  # [128, 2, IMG, HW]
        ci = c[core * IMG : (core + 1) * IMG]  # [IMG, 512]
        ci = np.ascontiguousarray(ci.T.reshape(4, P, IMG).transpose(1, 0, 2))
        in_maps.append(
            {
                "xin": xi,
                "wskip": wskip,
                "w0": w0p,
                "w1": w1p,
                "wqkv": wqkvT,
                "wproj": wprojT,
                "cvec": ci,
                "ones": ones,
            }
        )

    # The axon-tunneled device occasionally reports a transient
    # NRT_EXEC_UNIT_UNRECOVERABLE on the first execution after a fresh
    # process start; a retry succeeds.
    import time as _time

    res = None
    for attempt in range(5):
        try:
            res = run_bass_kernel_spmd(nc, in_maps, core_ids=list(range(N_CORES)))
            break
        except Exception:
            if attempt == 4:
                raise
            _time.sleep(2.0 * (attempt + 1))
    outs = []
    for core in range(N_CORES):
        o = res.results[core]["out"]  # [128, 4, IMG, HW]
        o = o.transpose(2, 1, 0, 3).reshape(IMG, 512, H, W)
        outs.append(o)
    return np.concatenate(outs, axis=0).astype(np.float32)

